# revision 35
# baseline (speedup 1.0000x reference)
"""Int4-quantized column-parallel linear (LLaMA-7B FFN up-proj) on 8 TRN2 cores.

y[b,s,o] = sum_i x[b,s,i] * (unpack_int4(weight_q)[o,i] * scale[o]) + bias[o]

Strategy (per core, 1/8 of out_features = 1376):
  - fp8 DoubleRow matmuls at 0.5 cycles/row (2x the fp16 rate). int4 weights
    are exact in fp8e4 (e4m3). x is decomposed into x_hi = Q8(x) plus
    x_lo = Q8(x - x_hi) ("double-fp8"); the hi pass covers all of K, the lo
    correction covers 12 of 16 k-tiles (measured end-to-end error ~1.3e-2,
    inside the 2e-2 gate), so each 256-wide k-tile costs 1.75 matmul rows
    instead of fp16's 2.
  - out_features ride the PSUM partition dim, so the per-channel scale/bias
    are per-partition scalars and the whole drain is one ACT activation
    (Identity with scale+bias APs). ACT uses Identity exclusively -> a
    single activation-table load for the whole kernel.
  - x and the packed weights are staged in DRAM K-major (host-side
    relayout only -- no values change), so the device never transposes:
    weights unpack straight to fp8 with two DVE shift ops per k-tile
    (shl 28/24 + sar 28 sign-extends the nibble), and x tiles DMA directly
    into [k, tok] layout for conversion.
  - the kernel returns y^T [feat, tok] per core; the host reassembles.
"""

from contextlib import ExitStack

import numpy as np

import concourse.bass as bass
import concourse.tile as tile
from concourse import bacc, mybir

F32 = mybir.dt.float32
F16 = mybir.dt.float16
F8 = mybir.dt.float8e4
I32 = mybir.dt.int32
I16 = mybir.dt.int16

B, S, IN, OUT = 4, 2048, 4096, 11008
NCORES = 8
TOK = B * S
FEAT = OUT // NCORES

P = 128
KB2 = IN // 256          # 16 DoubleRow k-tiles (256 contraction each)
KB2_LO = 10              # k-tiles that get the lo-pass correction
KP = IN // P             # 32 plain 128-k tiles
CHUNK = 512              # token chunk per PSUM sweep
NCHUNK = TOK // CHUNK    # 16
NSUB = CHUNK // P        # 4 conversion subtiles per chunk


def _feat_tiles(feat):
    out = []
    f0 = 0
    while f0 < feat:
        out.append((f0, min(P, feat - f0)))
        f0 += P
    return out


def build(tok=TOK, in_dim=IN, feat=FEAT):
    kb2 = in_dim // 256
    kp = in_dim // P
    nchunk = tok // CHUNK
    ftiles = _feat_tiles(feat)
    IDENT = mybir.ActivationFunctionType.Identity

    nc = bacc.Bacc("TRN2", target_bir_lowering=False, debug=False,
                   num_devices=NCORES)
    # xT: host-permuted K-major x. row r holds the in-feature matching the
    # nibble order the weight unpack produces below (see _x_row_permutation).
    xT_d = nc.dram_tensor("xT", [in_dim, tok], F32, kind="ExternalInput").ap()
    # wqT4: host-transposed packed weights, 8 nibbles per int32 [in//8, feat].
    wqT_d = nc.dram_tensor("wqT4", [in_dim // 8, feat], I32, kind="ExternalInput").ap()
    sc_d = nc.dram_tensor("scale", [feat], F32, kind="ExternalInput").ap()
    bi_d = nc.dram_tensor("bias", [feat], F32, kind="ExternalInput").ap()
    yT_d = nc.dram_tensor("yT", [feat, tok], F16, kind="ExternalOutput").ap()

    with tile.TileContext(nc) as tc, ExitStack() as ctx:
        const = ctx.enter_context(tc.tile_pool(name="const", bufs=1))
        wtp = ctx.enter_context(tc.tile_pool(name="wt", bufs=1))
        wqp = ctx.enter_context(tc.tile_pool(name="wqp", bufs=3))
        nibp = ctx.enter_context(tc.tile_pool(name="nibp", bufs=4))
        x32p = ctx.enter_context(tc.tile_pool(name="x32", bufs=4))
        x8p = ctx.enter_context(tc.tile_pool(name="x8", bufs=2))
        ysbp = ctx.enter_context(tc.tile_pool(name="ysb", bufs=3))
        pout = ctx.enter_context(tc.tile_pool(name="pout", bufs=8, space="PSUM"))

        # per-out-channel scale/bias as per-partition scalars [p, ftile]
        nfull = len([1 for _, fsz in ftiles if fsz == P])
        sc_t = const.tile([P, len(ftiles)], F32)
        bi_t = const.tile([P, len(ftiles)], F32)
        for vec_d, vec_t in ((sc_d, sc_t), (bi_d, bi_t)):
            nc.sync.dma_start(
                out=vec_t[:, :nfull],
                in_=bass.AP(tensor=vec_d.tensor, offset=vec_d.offset,
                            ap=[[1, P], [P, nfull]]),
            )
            f0, fsz = ftiles[-1]
            if fsz < P:
                nc.sync.dma_start(
                    out=vec_t[:fsz, nfull:],
                    in_=bass.AP(tensor=vec_d.tensor, offset=vec_d.offset + f0,
                                ap=[[1, fsz], [0, 1]]),
                )

        # Persistent dequantized fp8 weights: [in(part), kb2, s, feat]
        # slot s=0 <- low nibble (even in-feature), s=1 <- high nibble (odd).
        w8 = wtp.tile([P, kb2, 2, feat], F8)

        # First chunks are 256 tokens (cheap lead-in while the weight
        # unpack streams); larger groups amortize better once warm.
        chunks = [(i * 256, 256) for i in range(2)]
        t0 = chunks[-1][0] + 256
        while t0 < tok:
            chunks.append((t0, CHUNK))
            t0 += CHUNK

        state = {}

        def alloc_chunk(ci, t0, tlen):
            # Returns (dma_emitters, convert_emitters) for chunk ci.
            x8hi = x8p.tile([P, kb2, 2, tlen], F8, tag="hi")
            x8lo = x8p.tile([P, KB2_LO, 2, tlen], F8, tag="lo")
            state[ci] = (x8hi, x8lo)
            hiv = x8hi[:].rearrange("p a b t -> p (a b) t")
            lov = x8lo[:].rearrange("p a b t -> p (a b) t")
            x32s = [x32p.tile([P, kp, P], F32, name=f"x32_{ci}_{s}", tag="x32")
                    for s in range(tlen // P)]

            def emit_dma(s):
                nc.sync.dma_start(
                    out=x32s[s][:],
                    in_=bass.AP(tensor=xT_d.tensor,
                                offset=xT_d.offset + t0 + s * P,
                                ap=[[tok, P], [P * tok, kp], [1, P]]),
                )

            def emit_convert(s):
                ts = slice(s * P, (s + 1) * P)
                nc.scalar.activation(out=hiv[:, :, ts], in_=x32s[s][:],
                                     func=mybir.ActivationFunctionType.Identity)
                nc.vector.tensor_tensor(out=lov[:, :, ts],
                                        in0=x32s[s][:, :2 * KB2_LO, :],
                                        in1=hiv[:, :2 * KB2_LO, ts],
                                        op=mybir.AluOpType.subtract)

            nsub = tlen // P
            return ([lambda s=s: emit_dma(s) for s in range(nsub)],
                    [lambda s=s: emit_convert(s) for s in range(nsub)])

        def emit_chunk(ci, t0, tlen, inject):
            # inject: convert emitters for the NEXT chunk, run between
            # feature tiles so the ACT/DVE FIFOs stay unblocked.
            inject = list(inject)
            inject_after = {1, 3, 5, 7}
            x8hi, x8lo = state[ci]
            for fidx, (f0, fsz) in enumerate(ftiles):
                fi = f0 // P
                po = pout.tile([P, CHUNK], F32)
                for kk in range(kb2):
                    nc.tensor.matmul(
                        out=po[:fsz, :tlen],
                        lhsT=w8[:, kk, :, f0:f0 + fsz],
                        rhs=x8hi[:, kk, :, :],
                        start=(kk == 0),
                        stop=(kk == kb2 - 1),
                        perf_mode=mybir.MatmulPerfMode.DoubleRow,
                    )
                    if kk < KB2_LO:
                        nc.tensor.matmul(
                            out=po[:fsz, :tlen],
                            lhsT=w8[:, kk, :, f0:f0 + fsz],
                            rhs=x8lo[:, kk, :, :],
                            start=False,
                            stop=False,
                            perf_mode=mybir.MatmulPerfMode.DoubleRow,
                        )
                ysb = ysbp.tile([P, CHUNK], F16)
                nc.scalar.activation(
                    out=ysb[:fsz, :tlen], in_=po[:fsz, :tlen],
                    func=mybir.ActivationFunctionType.Identity,
                    scale=sc_t[:fsz, fi:fi + 1], bias=bi_t[:fsz, fi:fi + 1])
                nc.gpsimd.dma_start(
                    out=yT_d[f0:f0 + fsz, t0:t0 + tlen],
                    in_=ysb[:fsz, :tlen])
                if fidx in inject_after and inject:
                    inject.pop(0)()
            while inject:
                inject.pop(0)()
            del state[ci]

        def emit_wq_dma(jt):
            wq_t = wqp.tile([P, feat], I32, name=f"wq_t_{jt}", tag="wq")
            nc.sync.dma_start(out=wq_t[:], in_=wqT_d[jt * P:(jt + 1) * P])
            return wq_t

        njt = in_dim // 8 // P  # 4 wq DMA tiles
        # w8 viewed so kb2 = jt*4 + 2*h + mhalf (h = lo16/hi16 of each int32)
        w8v = w8[:].rearrange("p (jt h mh) s f -> p jt h mh s f", jt=njt, h=2)

        def emit_wq_unpack(jt, wq_t):
            # sign-extend each nibble via i32 shifts on DVE (bitvec ops can't
            # cast and the shift ISA is 32-bit only), cast on ACT;
            # int32 -> fp8e4 is exact in [-8, 7].
            for n in range(8):
                nib = nibp.tile([P, feat], I32, name=f"nib_{jt}_{n}",
                                tag="nib")
                nc.vector.tensor_scalar(
                    out=nib[:], in0=wq_t[:], scalar1=28 - 4 * n, scalar2=28,
                    op0=mybir.AluOpType.logical_shift_left,
                    op1=mybir.AluOpType.arith_shift_right)
                nc.scalar.activation(
                    out=w8[:, jt * 4 + n // 2, n % 2, :], in_=nib[:],
                    func=mybir.ActivationFunctionType.Identity)

        # ---- Phase W + startup, interleaved ----
        # The wq stream and the first chunks' x loads share the DMA engines;
        # weave them so the PE can trickle through kb2 tiles as weights land
        # while the first conversions complete early.
        dmas0, convs0 = alloc_chunk(0, *chunks[0])
        dmas1, convs1 = alloc_chunk(1, *chunks[1])
        startup_dmas = dmas0 + dmas1
        wq_ts = {}
        for i in range(max(njt, len(startup_dmas))):
            if i < len(startup_dmas):
                startup_dmas[i]()
            if i < njt:
                wq_ts[i] = emit_wq_dma(i)
            if i < len(convs0):
                convs0[i]()
        # All unpacks must precede the main loop (chunk 0's matmuls read
        # every kb2 tile); weave chunk 1's converts between them so they
        # don't queue behind all 16 casts on ACT.
        emit_wq_unpack(0, wq_ts[0])
        emit_wq_unpack(1, wq_ts[1])
        for em in convs1:
            em()
        emit_wq_unpack(2, wq_ts[2])
        emit_wq_unpack(3, wq_ts[3])
        convs1 = []

        # ---- Main loop: software-pipelined over token chunks ----
        for ci in range(len(chunks)):
            if ci == 0:
                convs = convs1
            elif ci + 1 < len(chunks):
                dmas, convs = alloc_chunk(ci + 1, *chunks[ci + 1])
                for em in dmas:
                    em()
            else:
                convs = []
            emit_chunk(ci, *chunks[ci], inject=convs)

    nc.compile()
    return nc


_CACHE = {}


def _get_program():
    if "nc" not in _CACHE:
        _CACHE["nc"] = build()
    return _CACHE["nc"]


def _x_row_permutation(in_dim=IN):
    # device x32 row r = (2*kb2 + s)*128 + p must hold in-feature
    # 8*(jt*128 + p) + 2*b + s  with kb2 = jt*4 + b  (8 nibbles per int32).
    r = np.arange(in_dim)
    kb2 = r // 256
    s = (r // 128) % 2
    p = r % 128
    return 8 * ((kb2 // 4) * 128 + p) + 2 * (kb2 % 4) + s


def _pack_wq(wq_slice):
    # [feat, in//2] int32 byte-pairs -> [in//8, feat] int32, 4 byte-pairs
    # (8 nibbles) per int32. Pure bit-layout change of the packed data.
    u8 = np.ascontiguousarray(wq_slice.T).astype(np.uint8)     # [in//2, feat]
    half, feat = u8.shape
    grp = np.ascontiguousarray(u8.reshape(half // 4, 4, feat).transpose(0, 2, 1))
    return grp.view(np.int32).reshape(half // 4, feat)


def kernel(x, weight_q, scale, bias):
    from concourse.bass_utils import run_bass_kernel_spmd

    try:
        import jax

        jax.config.update("jax_compilation_cache_dir", "/root/problem/jax_cache")
        jax.config.update("jax_persistent_cache_min_compile_time_secs", 0)
    except Exception:
        pass

    nc = _get_program()
    xr = np.asarray(x, dtype=np.float32).reshape(TOK, IN)
    xT = np.ascontiguousarray(xr.T[_x_row_permutation()])
    wq = np.asarray(weight_q, dtype=np.int32)
    sc = np.asarray(scale, dtype=np.float32)
    bi = np.asarray(bias, dtype=np.float32)
    in_maps = []
    for c in range(NCORES):
        f0 = c * FEAT
        in_maps.append({
            "xT": xT,
            "wqT4": _pack_wq(wq[f0:f0 + FEAT]),
            "scale": np.ascontiguousarray(sc[f0:f0 + FEAT]),
            "bias": np.ascontiguousarray(bi[f0:f0 + FEAT]),
        })
    res = run_bass_kernel_spmd(nc, in_maps, list(range(NCORES))).results
    y = np.empty((TOK, OUT), dtype=np.float32)
    for c in range(NCORES):
        f0 = c * FEAT
        y[:, f0:f0 + FEAT] = res[c]["yT"].T.astype(np.float32)
    return y.reshape(B, S, OUT)


# revision 37
# speedup vs baseline: 1.0002x; 1.0002x over previous
"""Int4-quantized column-parallel linear (LLaMA-7B FFN up-proj) on 8 TRN2 cores.

y[b,s,o] = sum_i x[b,s,i] * (unpack_int4(weight_q)[o,i] * scale[o]) + bias[o]

Strategy (per core, 1/8 of out_features = 1376):
  - fp8 DoubleRow matmuls at 0.5 cycles/row (2x the fp16 rate). int4 weights
    are exact in fp8e4 (e4m3). x is decomposed into x_hi = Q8(x) plus
    x_lo = Q8(x - x_hi) ("double-fp8"); the hi pass covers all of K, the lo
    correction covers 10 of 16 k-tiles (measured end-to-end error 1.63e-2,
    inside the 2e-2 gate), so each 256-wide k-tile costs 1.625 matmul rows
    instead of fp16's 2.
  - out_features ride the PSUM partition dim, so the per-channel scale/bias
    are per-partition scalars and the whole drain is one ACT activation
    (Identity with scale+bias APs). ACT uses Identity exclusively -> a
    single activation-table load for the whole kernel. y is stored fp16
    (0.03% rounding) to halve the output DMA.
  - x and the packed weights are staged in DRAM K-major (host-side
    relayout only -- no values change; the packed int4 bytes are just
    regrouped 8-nibbles-per-int32), so the device never transposes:
    weights unpack straight to fp8 via DVE shift pairs (shl/sar
    sign-extends each nibble) + ACT casts, and x tiles DMA directly into
    [k, tok] layout for conversion. y saves ride the idle GpSimd queue.
  - emission is software-pipelined: the next chunk's x DMAs issue before
    the current chunk's matmuls, conversions are interleaved between
    feature tiles, and the weight unpack is woven through the first
    chunks' loads -> zero PE gaps in steady state.
  - the kernel returns y^T [feat, tok] per core; the host reassembles.
"""

from contextlib import ExitStack

import numpy as np

import concourse.bass as bass
import concourse.tile as tile
from concourse import bacc, mybir

F32 = mybir.dt.float32
F16 = mybir.dt.float16
F8 = mybir.dt.float8e4
I32 = mybir.dt.int32
I16 = mybir.dt.int16

B, S, IN, OUT = 4, 2048, 4096, 11008
NCORES = 8
TOK = B * S
FEAT = OUT // NCORES

P = 128
KB2 = IN // 256          # 16 DoubleRow k-tiles (256 contraction each)
KB2_LO = 10              # k-tiles that get the lo-pass correction
KP = IN // P             # 32 plain 128-k tiles
CHUNK = 512              # token chunk per PSUM sweep
NCHUNK = TOK // CHUNK    # 16
NSUB = CHUNK // P        # 4 conversion subtiles per chunk


def _feat_tiles(feat):
    out = []
    f0 = 0
    while f0 < feat:
        out.append((f0, min(P, feat - f0)))
        f0 += P
    return out


def build(tok=TOK, in_dim=IN, feat=FEAT):
    kb2 = in_dim // 256
    kp = in_dim // P
    nchunk = tok // CHUNK
    ftiles = _feat_tiles(feat)
    IDENT = mybir.ActivationFunctionType.Identity

    nc = bacc.Bacc("TRN2", target_bir_lowering=False, debug=False,
                   num_devices=NCORES)
    # xT: host-permuted K-major x. row r holds the in-feature matching the
    # nibble order the weight unpack produces below (see _x_row_permutation).
    xT_d = nc.dram_tensor("xT", [in_dim, tok], F32, kind="ExternalInput").ap()
    # wqT4: host-transposed packed weights, 8 nibbles per int32 [in//8, feat].
    wqT_d = nc.dram_tensor("wqT4", [in_dim // 8, feat], I32, kind="ExternalInput").ap()
    sc_d = nc.dram_tensor("scale", [feat], F32, kind="ExternalInput").ap()
    bi_d = nc.dram_tensor("bias", [feat], F32, kind="ExternalInput").ap()
    yT_d = nc.dram_tensor("yT", [feat, tok], F16, kind="ExternalOutput").ap()

    with tile.TileContext(nc) as tc, ExitStack() as ctx:
        const = ctx.enter_context(tc.tile_pool(name="const", bufs=1))
        wtp = ctx.enter_context(tc.tile_pool(name="wt", bufs=1))
        wqp = ctx.enter_context(tc.tile_pool(name="wqp", bufs=3))
        nibp = ctx.enter_context(tc.tile_pool(name="nibp", bufs=4))
        x32p = ctx.enter_context(tc.tile_pool(name="x32", bufs=4))
        x8p = ctx.enter_context(tc.tile_pool(name="x8", bufs=2))
        ysbp = ctx.enter_context(tc.tile_pool(name="ysb", bufs=3))
        pout = ctx.enter_context(tc.tile_pool(name="pout", bufs=8, space="PSUM"))

        # per-out-channel scale/bias as per-partition scalars [p, ftile]
        nfull = len([1 for _, fsz in ftiles if fsz == P])
        sc_t = const.tile([P, len(ftiles)], F32)
        bi_t = const.tile([P, len(ftiles)], F32)
        for vec_d, vec_t in ((sc_d, sc_t), (bi_d, bi_t)):
            nc.sync.dma_start(
                out=vec_t[:, :nfull],
                in_=bass.AP(tensor=vec_d.tensor, offset=vec_d.offset,
                            ap=[[1, P], [P, nfull]]),
            )
            f0, fsz = ftiles[-1]
            if fsz < P:
                nc.sync.dma_start(
                    out=vec_t[:fsz, nfull:],
                    in_=bass.AP(tensor=vec_d.tensor, offset=vec_d.offset + f0,
                                ap=[[1, fsz], [0, 1]]),
                )

        # Persistent dequantized fp8 weights: [in(part), kb2, s, feat]
        # slot s=0 <- low nibble (even in-feature), s=1 <- high nibble (odd).
        w8 = wtp.tile([P, kb2, 2, feat], F8)

        # First chunks are 256 tokens (cheap lead-in while the weight
        # unpack streams); larger groups amortize better once warm.
        chunks = [(i * 256, 256) for i in range(6)]
        t0 = chunks[-1][0] + 256
        while t0 < tok:
            chunks.append((t0, CHUNK))
            t0 += CHUNK

        state = {}

        def alloc_chunk(ci, t0, tlen):
            # Returns (dma_emitters, convert_emitters) for chunk ci.
            x8hi = x8p.tile([P, kb2, 2, tlen], F8, tag="hi")
            x8lo = x8p.tile([P, KB2_LO, 2, tlen], F8, tag="lo")
            state[ci] = (x8hi, x8lo)
            hiv = x8hi[:].rearrange("p a b t -> p (a b) t")
            lov = x8lo[:].rearrange("p a b t -> p (a b) t")
            x32s = [x32p.tile([P, kp, P], F32, name=f"x32_{ci}_{s}", tag="x32")
                    for s in range(tlen // P)]

            def emit_dma(s):
                nc.sync.dma_start(
                    out=x32s[s][:],
                    in_=bass.AP(tensor=xT_d.tensor,
                                offset=xT_d.offset + t0 + s * P,
                                ap=[[tok, P], [P * tok, kp], [1, P]]),
                )

            def emit_convert(s):
                ts = slice(s * P, (s + 1) * P)
                nc.scalar.activation(out=hiv[:, :, ts], in_=x32s[s][:],
                                     func=mybir.ActivationFunctionType.Identity)
                nc.vector.tensor_tensor(out=lov[:, :, ts],
                                        in0=x32s[s][:, :2 * KB2_LO, :],
                                        in1=hiv[:, :2 * KB2_LO, ts],
                                        op=mybir.AluOpType.subtract)

            nsub = tlen // P
            return ([lambda s=s: emit_dma(s) for s in range(nsub)],
                    [lambda s=s: emit_convert(s) for s in range(nsub)])

        def emit_chunk(ci, t0, tlen, inject):
            # inject: convert emitters for the NEXT chunk, run between
            # feature tiles so the ACT/DVE FIFOs stay unblocked.
            inject = list(inject)
            inject_after = {1, 3, 5, 7}
            x8hi, x8lo = state[ci]
            for fidx, (f0, fsz) in enumerate(ftiles):
                fi = f0 // P
                po = pout.tile([P, CHUNK], F32)
                for kk in range(kb2):
                    nc.tensor.matmul(
                        out=po[:fsz, :tlen],
                        lhsT=w8[:, kk, :, f0:f0 + fsz],
                        rhs=x8hi[:, kk, :, :],
                        start=(kk == 0),
                        stop=(kk == kb2 - 1),
                        perf_mode=mybir.MatmulPerfMode.DoubleRow,
                    )
                    if kk < KB2_LO:
                        nc.tensor.matmul(
                            out=po[:fsz, :tlen],
                            lhsT=w8[:, kk, :, f0:f0 + fsz],
                            rhs=x8lo[:, kk, :, :],
                            start=False,
                            stop=False,
                            perf_mode=mybir.MatmulPerfMode.DoubleRow,
                        )
                ysb = ysbp.tile([P, CHUNK], F16)
                nc.scalar.activation(
                    out=ysb[:fsz, :tlen], in_=po[:fsz, :tlen],
                    func=mybir.ActivationFunctionType.Identity,
                    scale=sc_t[:fsz, fi:fi + 1], bias=bi_t[:fsz, fi:fi + 1])
                nc.gpsimd.dma_start(
                    out=yT_d[f0:f0 + fsz, t0:t0 + tlen],
                    in_=ysb[:fsz, :tlen])
                if fidx in inject_after and inject:
                    inject.pop(0)()
            while inject:
                inject.pop(0)()
            del state[ci]

        def emit_wq_dma(jt):
            wq_t = wqp.tile([P, feat], I32, name=f"wq_t_{jt}", tag="wq")
            nc.sync.dma_start(out=wq_t[:], in_=wqT_d[jt * P:(jt + 1) * P])
            return wq_t

        njt = in_dim // 8 // P  # 4 wq DMA tiles
        # w8 viewed so kb2 = jt*4 + 2*h + mhalf (h = lo16/hi16 of each int32)
        w8v = w8[:].rearrange("p (jt h mh) s f -> p jt h mh s f", jt=njt, h=2)

        def emit_wq_unpack(jt, wq_t):
            # sign-extend each nibble via i32 shifts on DVE (bitvec ops can't
            # cast and the shift ISA is 32-bit only), cast on ACT;
            # int32 -> fp8e4 is exact in [-8, 7].
            for n in range(8):
                nib = nibp.tile([P, feat], I32, name=f"nib_{jt}_{n}",
                                tag="nib")
                nc.vector.tensor_scalar(
                    out=nib[:], in0=wq_t[:], scalar1=28 - 4 * n, scalar2=28,
                    op0=mybir.AluOpType.logical_shift_left,
                    op1=mybir.AluOpType.arith_shift_right)
                nc.scalar.activation(
                    out=w8[:, jt * 4 + n // 2, n % 2, :], in_=nib[:],
                    func=mybir.ActivationFunctionType.Identity)

        # ---- Phase W + startup, interleaved ----
        # The wq stream and the first chunks' x loads share the DMA engines;
        # weave them so the PE can trickle through kb2 tiles as weights land
        # while the first conversions complete early.
        dmas0, convs0 = alloc_chunk(0, *chunks[0])
        dmas1, convs1 = alloc_chunk(1, *chunks[1])
        startup_dmas = dmas0 + dmas1
        wq_ts = {}
        for i in range(max(njt, len(startup_dmas))):
            if i < len(startup_dmas):
                startup_dmas[i]()
            if i < njt:
                wq_ts[i] = emit_wq_dma(i)
            if i < len(convs0):
                convs0[i]()
        # All unpacks must precede the main loop (chunk 0's matmuls read
        # every kb2 tile); weave chunk 1's converts between them so they
        # don't queue behind all 16 casts on ACT.
        emit_wq_unpack(0, wq_ts[0])
        emit_wq_unpack(1, wq_ts[1])
        for em in convs1:
            em()
        emit_wq_unpack(2, wq_ts[2])
        emit_wq_unpack(3, wq_ts[3])
        convs1 = []

        # ---- Main loop: software-pipelined over token chunks ----
        for ci in range(len(chunks)):
            if ci == 0:
                convs = convs1
            elif ci + 1 < len(chunks):
                dmas, convs = alloc_chunk(ci + 1, *chunks[ci + 1])
                for em in dmas:
                    em()
            else:
                convs = []
            emit_chunk(ci, *chunks[ci], inject=convs)

    nc.compile()
    return nc


_CACHE = {}


def _get_program():
    if "nc" not in _CACHE:
        _CACHE["nc"] = build()
    return _CACHE["nc"]


def _x_row_permutation(in_dim=IN):
    # device x32 row r = (2*kb2 + s)*128 + p must hold in-feature
    # 8*(jt*128 + p) + 2*b + s  with kb2 = jt*4 + b  (8 nibbles per int32).
    r = np.arange(in_dim)
    kb2 = r // 256
    s = (r // 128) % 2
    p = r % 128
    return 8 * ((kb2 // 4) * 128 + p) + 2 * (kb2 % 4) + s


def _pack_wq(wq_slice):
    # [feat, in//2] int32 byte-pairs -> [in//8, feat] int32, 4 byte-pairs
    # (8 nibbles) per int32. Pure bit-layout change of the packed data.
    u8 = np.ascontiguousarray(wq_slice.T).astype(np.uint8)     # [in//2, feat]
    half, feat = u8.shape
    grp = np.ascontiguousarray(u8.reshape(half // 4, 4, feat).transpose(0, 2, 1))
    return grp.view(np.int32).reshape(half // 4, feat)


def kernel(x, weight_q, scale, bias):
    from concourse.bass_utils import run_bass_kernel_spmd

    try:
        import jax

        jax.config.update("jax_compilation_cache_dir", "/root/problem/jax_cache")
        jax.config.update("jax_persistent_cache_min_compile_time_secs", 0)
    except Exception:
        pass

    nc = _get_program()
    xr = np.asarray(x, dtype=np.float32).reshape(TOK, IN)
    xT = np.ascontiguousarray(xr.T[_x_row_permutation()])
    wq = np.asarray(weight_q, dtype=np.int32)
    sc = np.asarray(scale, dtype=np.float32)
    bi = np.asarray(bias, dtype=np.float32)
    in_maps = []
    for c in range(NCORES):
        f0 = c * FEAT
        in_maps.append({
            "xT": xT,
            "wqT4": _pack_wq(wq[f0:f0 + FEAT]),
            "scale": np.ascontiguousarray(sc[f0:f0 + FEAT]),
            "bias": np.ascontiguousarray(bi[f0:f0 + FEAT]),
        })
    res = run_bass_kernel_spmd(nc, in_maps, list(range(NCORES))).results
    y = np.empty((TOK, OUT), dtype=np.float32)
    for c in range(NCORES):
        f0 = c * FEAT
        y[:, f0:f0 + FEAT] = res[c]["yT"].T.astype(np.float32)
    return y.reshape(B, S, OUT)
